# revision 5
# baseline (speedup 1.0000x reference)
"""Trainium2 Bass kernel for nn_Attention_47261820125787.

Full multi-head attention layer with low-rank-reconstructed projection
weights (w = LM @ RM + W), B=16, S=1024, H=1024, 16 heads x 64 dim.

Sharding: data-parallel over batch -- each of the 8 cores processes 2
batches with the full set of weights. No collectives.

Host-side prep is layout-only: weights are passed pre-transposed
(W.T, LM.T) and hidden_states is passed as [B, H, S] so that every DMA
is contiguous; all math (weight reconstruction, projections, attention)
runs on-device in fp32r.

On-device dataflow per core:
  wT = (LM@RM).T + W.T          reconstructed per projection in SBUF
  qT, kT = [o, s] layouts       (lhsT = wT tiles, rhs = xT tiles)
  v     = [s, o] layout         (lhsT = xT tiles, rhs = wT tiles),
          stored with a ones-column interleaved per head so the ctx
          matmul also produces softmax denominators for free
  E^T   = exp(scores^T / 8)     scores computed transposed [sk, sq];
          no max-subtraction (scores ~ N(0,1), exp can't overflow)
  ctx^T = (v|1).T @ E           [d+1, sq] per head; row 64 = sum_k E
  ctx normalized by 1/sums (vector.reciprocal + gpsimd partition
  broadcast), out = ctx @ wo.T + bo evicted in natural [s, o] layout.
"""

import numpy as np
from contextlib import ExitStack

import concourse.bass as bass
import concourse.tile as tile
from concourse import bacc, mybir
from concourse.bass_utils import run_bass_kernel_spmd

B, S, H, NH, HD = 16, 1024, 1024, 16, 64
KR = 64             # low-rank dim
N_CORES = 8
BPC = B // N_CORES  # batches per core

f32 = mybir.dt.float32
f32r = mybir.dt.float32r
AF = mybir.ActivationFunctionType
ALU = mybir.AluOpType

PROJS = ("q", "k", "v", "o")
NT = H // 128       # 8 partition tiles over hidden dim
NSC = S // 512      # 2 free chunks over sequence
VW = NH * (HD + 1)  # v_store width: 64 cols + 1 ones col per head


def _emit(ctx: ExitStack, tc: tile.TileContext, d: dict):
    nc = tc.nc

    # ---------------- constants (resident all kernel) ----------------
    cpool = ctx.enter_context(tc.tile_pool(name="consts", bufs=1))

    bcol = {}
    for p in ("q", "k"):
        t = cpool.tile([128, NT], f32, name=f"bcol_{p}")
        nc.sync.dma_start(t[:], d[f"BCOL{p}"][:])
        bcol[p] = t

    bb = {}
    for p in ("v", "o"):
        row = cpool.tile([1, H], f32, name=f"brow_{p}")
        nc.sync.dma_start(row[:], d[f"BROW{p}"][:])
        t = cpool.tile([128, H], f32, name=f"bb_{p}")
        nc.gpsimd.partition_broadcast(t[:], row[:])
        bb[p] = t

    # -------- per-batch persistent activations (q/k/v stores) --------
    bpool = ctx.enter_context(tc.tile_pool(name="acts", bufs=1))
    qT = [bpool.tile([128, S], f32r, name=f"qT{i}") for i in range(NT)]
    kT = [bpool.tile([128, S], f32r, name=f"kT{i}") for i in range(NT)]
    vs = [bpool.tile([128, VW], f32r, name=f"vs{i}") for i in range(NT)]

    psum = ctx.enter_context(tc.tile_pool(name="psum", bufs=1, space="PSUM"))

    def recon_wt(p, pool, wt):
        """wt[i][:, :] = ((LM@RM).T + W.T)[128i:128(i+1), :] in f32r."""
        lmt = pool.tile([64, H], f32r, tag="lmt", name=f"lmt_{p}")
        nc.sync.dma_start(lmt[:], d[f"LMT{p}"][:])
        rm = pool.tile([64, H], f32r, tag="rm", name=f"rm_{p}")
        nc.sync.dma_start(rm[:], d[f"RM{p}"][:])
        for i in range(NT):
            for oc in range(NSC):
                pw = psum.tile([128, 512], f32, tag="proj", bufs=3, name=f"pw_{p}_{i}_{oc}")
                nc.tensor.matmul(
                    pw[:],
                    rm[:, i * 128:(i + 1) * 128],
                    lmt[:, oc * 512:(oc + 1) * 512],
                    start=True, stop=True,
                )
                wsrc = pool.tile([128, 512], f32, tag="wsrc", bufs=3, name=f"wsrc_{p}_{i}_{oc}")
                nc.sync.dma_start(
                    wsrc[:], d[f"WT{p}"][i * 128:(i + 1) * 128,
                                         oc * 512:(oc + 1) * 512])
                nc.vector.tensor_tensor(
                    wt[i][:, oc * 512:(oc + 1) * 512], pw[:], wsrc[:], ALU.add)

    # ================= per batch =================
    for b in range(BPC):
        # ---- load xT; reconstruct + apply q/k/v projections ----
        with tc.tile_pool(name=f"xw{b}", bufs=1) as pxw:
            xt = [pxw.tile([128, S], f32r, name=f"xt{b}_{i}")
                  for i in range(NT)]
            for i in range(NT):
                nc.sync.dma_start(
                    xt[i][:], d["xT"][b, i * 128:(i + 1) * 128, :])
            wt = [pxw.tile([128, H], f32r, name=f"wt{b}_{i}")
                  for i in range(NT)]

            for p, store in (("q", qT), ("k", kT)):
                recon_wt(p, pxw, wt)
                for ot in range(NT):
                    ps = [psum.tile([128, 512], f32, tag="proj", bufs=3,
                                    name=f"ps_{p}{b}_{ot}_{i}")
                          for i in range(NSC)]
                    for it in range(NT):
                        for sc in range(NSC):
                            nc.tensor.matmul(
                                ps[sc][:],
                                wt[it][:, ot * 128:(ot + 1) * 128],
                                xt[it][:, sc * 512:(sc + 1) * 512],
                                start=(it == 0), stop=(it == NT - 1),
                            )
                    for sc in range(NSC):
                        nc.vector.tensor_scalar_add(
                            store[ot][:, sc * 512:(sc + 1) * 512],
                            ps[sc][:], bcol[p][:, ot:ot + 1])

            # ---- v in natural [s, o] layout with interleaved ones ----
            recon_wt("v", pxw, wt)
            for st in range(NT):
                grp = vs[st][:].rearrange("p (h d) -> p h d", d=HD + 1)
                grp_f32 = vs[st][:].bitcast(f32).rearrange(
                    "p (h d) -> p h d", d=HD + 1)
                nc.vector.memset(grp_f32[:, :, HD:HD + 1], 1.0)
                ps = [psum.tile([128, 512], f32, tag="proj", bufs=3,
                                name=f"ps_v{b}_{st}_{i}")
                      for i in range(NSC)]
                for it in range(NT):
                    for oc in range(NSC):
                        nc.tensor.matmul(
                            ps[oc][:],
                            xt[it][:, st * 128:(st + 1) * 128],
                            wt[it][:, oc * 512:(oc + 1) * 512],
                            start=(it == 0), stop=(it == NT - 1),
                        )
                for oc in range(NSC):
                    dst = grp[:, oc * 8:(oc + 1) * 8, 0:HD]
                    nc.vector.tensor_tensor(
                        dst, ps[oc][:],
                        bb["v"][:, oc * 512:(oc + 1) * 512], ALU.add)

        with tc.tile_pool(name=f"ct{b}", bufs=1) as pct:
            cT = [pct.tile([128, S], f32r, name=f"cT{b}_{i}")
                  for i in range(NT)]

            # ---- attention ----
            with tc.tile_pool(name=f"att{b}", bufs=1) as patt:
                for h in range(NH):
                    ht, hp = divmod(h, 2)
                    hp *= 64
                    for sc in range(NSC):
                        et = []
                        for kt in range(NT):
                            pssc = psum.tile([128, 512], f32, tag="att",
                                             bufs=3,
                                             name=f"pssc{b}_{h}_{sc}_{kt}")
                            nc.tensor.matmul(
                                pssc[:],
                                kT[ht][hp:hp + 64, kt * 128:(kt + 1) * 128],
                                qT[ht][hp:hp + 64, sc * 512:(sc + 1) * 512],
                                start=True, stop=True,
                            )
                            e = patt.tile([128, 512], f32r, tag="E", bufs=11, name=f"e{b}_{h}_{sc}_{kt}")
                            nc.scalar.activation(e[:], pssc[:], AF.Exp,
                                                 scale=0.125)
                            et.append(e)
                        pc = psum.tile([65, 512], f32, tag="ctx", bufs=2, name=f"pc{b}_{h}_{sc}")
                        for kt in range(NT):
                            nc.tensor.matmul(
                                pc[:],
                                vs[kt][:, h * (HD + 1):(h + 1) * (HD + 1)],
                                et[kt][:],
                                start=(kt == 0), stop=(kt == NT - 1),
                            )
                        recip = patt.tile([1, 512], f32, tag="recip", bufs=3, name=f"recip{b}_{h}_{sc}")
                        nc.vector.reciprocal(recip[:], pc[64:65, :])
                        rb = patt.tile([64, 512], f32, tag="rb", bufs=3, name=f"rb{b}_{h}_{sc}")
                        nc.gpsimd.partition_broadcast(rb[:], recip[:])
                        nc.vector.tensor_tensor(
                            cT[ht][hp:hp + 64, sc * 512:(sc + 1) * 512],
                            pc[0:64, :], rb[:], ALU.mult)

            # ---- out projection ----
            with tc.tile_pool(name=f"wo{b}", bufs=1) as pwo:
                wt = [pwo.tile([128, H], f32r, name=f"wto{b}_{i}")
                      for i in range(NT)]
                recon_wt("o", pwo, wt)
                for st in range(NT):
                    ps = [psum.tile([128, 512], f32, tag="proj", bufs=3,
                                    name=f"ps_{p}{b}_{ot}_{i}")
                          for i in range(NSC)]
                    for it in range(NT):
                        for oc in range(NSC):
                            nc.tensor.matmul(
                                ps[oc][:],
                                cT[it][:, st * 128:(st + 1) * 128],
                                wt[it][:, oc * 512:(oc + 1) * 512],
                                start=(it == 0), stop=(it == NT - 1),
                            )
                    for oc in range(NSC):
                        osb = pwo.tile([128, 512], f32, tag="osb", bufs=3, name=f"osb{b}_{st}_{oc}")
                        nc.vector.tensor_tensor(
                            osb[:], ps[oc][:],
                            bb["o"][:, oc * 512:(oc + 1) * 512], ALU.add)
                        nc.sync.dma_start(
                            d["out"][b, st * 128:(st + 1) * 128,
                                     oc * 512:(oc + 1) * 512], osb[:])


def build_nc():
    nc = bacc.Bacc("TRN2", target_bir_lowering=False, debug=False,
                   num_devices=N_CORES)
    d = {}
    d["xT"] = nc.dram_tensor("xT", [BPC, H, S], f32r,
                             kind="ExternalInput").ap()
    for p in PROJS:
        d[f"WT{p}"] = nc.dram_tensor(f"WT{p}", [H, H], f32,
                                     kind="ExternalInput").ap()
        d[f"LMT{p}"] = nc.dram_tensor(f"LMT{p}", [KR, H], f32r,
                                      kind="ExternalInput").ap()
        d[f"RM{p}"] = nc.dram_tensor(f"RM{p}", [KR, H], f32r,
                                     kind="ExternalInput").ap()
    for p in ("q", "k"):
        d[f"BCOL{p}"] = nc.dram_tensor(f"BCOL{p}", [128, NT], f32,
                                       kind="ExternalInput").ap()
    for p in ("v", "o"):
        d[f"BROW{p}"] = nc.dram_tensor(f"BROW{p}", [1, H], f32,
                                       kind="ExternalInput").ap()
    d["out"] = nc.dram_tensor("out", [BPC, S, H], f32,
                              kind="ExternalOutput").ap()

    with tile.TileContext(nc) as tc, ExitStack() as ctx:
        _emit(ctx, tc, d)
    nc.compile()
    return nc


_CACHE = {}


def _prep_inputs(inputs):
    """Host-side, layout-only: transposes + slicing per core."""
    g = {k: np.asarray(v, dtype=np.float32) for k, v in inputs.items()
         if k != "task"}
    shared = {}
    for p in PROJS:
        WT = np.ascontiguousarray(g["W" + p].T)
        LMT = np.ascontiguousarray(g["LM" + p].T)
        F = g["F" + p]
        if not np.all(F == 1.0):
            # fold the per-output-channel SFG scale into the transposed
            # weights (identity in practice: F is spec'd all-ones)
            WT = WT * F
            LMT = np.ascontiguousarray(LMT * F)
        shared[f"WT{p}"] = WT
        shared[f"LMT{p}"] = LMT
        shared[f"RM{p}"] = np.ascontiguousarray(g["RM" + p])
    for p in ("q", "k"):
        shared[f"BCOL{p}"] = np.ascontiguousarray(
            (g["b" + p] * g["F" + p]).reshape(NT, 128).T)
    for p in ("v", "o"):
        shared[f"BROW{p}"] = np.ascontiguousarray(
            (g["b" + p] * g["F" + p]).reshape(1, H))
    hs = g["hidden_states"]
    in_maps = []
    for c in range(N_CORES):
        m = dict(shared)
        m["xT"] = np.ascontiguousarray(
            hs[c * BPC:(c + 1) * BPC].transpose(0, 2, 1))
        in_maps.append(m)
    return in_maps


def kernel(**inputs):
    if "nc" not in _CACHE:
        _CACHE["nc"] = build_nc()
    nc = _CACHE["nc"]
    in_maps = _prep_inputs(inputs)
    res = run_bass_kernel_spmd(nc, in_maps, list(range(N_CORES)))
    return np.concatenate([r["out"] for r in res.results], axis=0)
